# revision 16
# baseline (speedup 1.0000x reference)
"""Trainium2 Bass kernel for batched shared-query attention.

Problem:
  query [S=128, D=64] shared across all (b, w);
  keys/values [B=64, W=32, T=256, D=64];
  out[b, w] = softmax(query @ keys[b, w].T, axis=-1) @ values[b, w].

Strategy (8 NeuronCores, data-parallel over B), v5 = fp16 score matmuls,
bf16 output matmuls, 512B-chunk loads, engine-balanced elementwise work,
software-pipelined emission:
  Each core gets B_PER=8 batches (256 (b, w) pairs), G=4 pairs per group.
    1. K and V both loaded t-pair-interleaved: sbuf [128, G*128] fp32,
       partition p holds rows t=2p, 2p+1 (512B contiguous DMA chunks --
       512B packets run ~18 B/ns/engine vs ~15 for 256B).
    2. V fp32 -> bf16 cast into [128, G*2*65] with a ones column per
       (pair, t-parity) block (the 65th rhs column makes the output
       matmul emit the softmax denominator for free).  Cast is split:
       pairs 01 on GpSimd (its only possible job -- it cannot touch
       PSUM), pairs 23 on DVE.
    3. PE transpose of K (fp32) -> kt PSUM; PSUM -> SBUF fp16 copy split
       half on DVE / half on ACT (stacked Kt: partitions 0:64 = K^T of
       even t, 64:128 = odd t).  fp16 (not bf16): the 10-bit mantissa
       keeps softmax weight error ~0.4% (bf16 scores fail the 2e-2 gate).
    4. pT = stacked-Kt.T @ qz_cat, ONE fp16 matmul per pair (N=256,
       1 cyc/row): qz_cat [128, 256] fp16 holds Qt twice, zero-padded
       parity-blocked, so both t-parities come out side by side.
    5. ACT exp: ONE plain [128, 1024] PSUM -> SBUF bf16 instr per group
       (bf16 output: exp range exceeds fp16 max).
    6. Output matmuls, m=128: per (pair, t-parity j):
       out[s, v|den] += E_j.T @ [V_j | 1] (N=65, bf16), 8 per group into
       one PSUM tile [128, 260] with column-disjoint [128, 65] regions
       (single 2KB bank; first/last matmul carries start/stop).
    7. DVE reciprocal of the 4 denominators + one broadcast multiply;
       store DMA (256B chunks -- the s-major layout costs ~3% DMA time,
       much less than the extra matmuls/normalize of a 512B-store layout).
  Emission is software-pipelined across groups (loads at i, stages at
  i-1/i-2/i-3, store at i-5) so every engine's queue head is always
  ready: the PE never stalls (keeps its p-state ramped) and the store's
  semaphore wait cannot delay the next loads on the SP queue.
  All matmuls keep full 128-row contractions at tile_position (0,0) --
  alternating row-group (K=64) matmuls fault on HW.
"""

import sys

sys.path.insert(0, "/opt/trn_rl_repo")

import numpy as np

import concourse.bass as bass
from concourse import bacc
import concourse.mybir as mybir
import concourse.tile as tile
from concourse.bass_utils import run_bass_kernel_spmd
from concourse.masks import make_identity

F32 = mybir.dt.float32
BF16 = mybir.dt.bfloat16
FP16 = mybir.dt.float16
N_CORES = 8
B, W, T, S, D = 64, 32, 256, 128, 64
B_PER = B // N_CORES
G = 4  # (b, w) pairs per group


def build_bass(b_per=B_PER, w=W):
    nc = bacc.Bacc()
    q_t = nc.declare_dram_parameter("query", [S, D], F32, isOutput=False)
    k_t = nc.declare_dram_parameter("keys", [b_per, w, T, D], F32, isOutput=False)
    v_t = nc.declare_dram_parameter("values", [b_per, w, T, D], F32, isOutput=False)
    o_t = nc.declare_dram_parameter("out", [b_per, w, S, D], F32, isOutput=True)

    EXP = mybir.ActivationFunctionType.Exp

    with tile.TileContext(nc) as tc:
        with tc.tile_pool(name="const", bufs=1) as const:
            ident = const.tile([128, 128], F32)
            make_identity(nc, ident[:])
            q_sb = const.tile([S, D], F32)
            nc.sync.dma_start(out=q_sb[:], in_=q_t[:, :])
            # qz_cat [128, 256] fp16:
            #   rows 0:64,   cols   0:128 = Qt  (contracts Kt_even rows)
            #   rows 64:128, cols 128:256 = Qt  (contracts Kt_odd rows)
            #   everything else 0.
            qz_cat = const.tile([128, 2 * S], FP16)
            nc.vector.memset(qz_cat[:], 0.0)
            with tc.tile_pool(name="psetup", bufs=1, space="PSUM") as psetup:
                qt_ps = psetup.tile([64, S], F32)
                nc.tensor.matmul(
                    qt_ps[:, :], q_sb[:], ident[:],
                    is_transpose=True, start=True, stop=True,
                )
                nc.scalar.copy(qz_cat[0:64, 0:S], qt_ps[:])
            # place Qt on partitions 64:128 via a DRAM roundtrip
            # (cross-partition engine copies are not available)
            qt_scratch = nc.dram_tensor("qt_scratch", [64, S], FP16)
            nc.sync.dma_start(out=qt_scratch[:, :], in_=qz_cat[0:64, 0:S])
            nc.sync.dma_start(out=qz_cat[64:128, S : 2 * S], in_=qt_scratch[:, :])

            n_groups = b_per * (w // G)

            with (
                tc.tile_pool(name="kc", bufs=4) as kc_pool,
                tc.tile_pool(name="vf", bufs=4) as vf_pool,
                tc.tile_pool(name="vx", bufs=4) as vx_pool,
                tc.tile_pool(name="kts", bufs=3) as kt_pool,
                tc.tile_pool(name="et", bufs=3) as et_pool,
                tc.tile_pool(name="osb", bufs=4) as os_pool,
                tc.tile_pool(name="rc", bufs=4) as rc_pool,
                tc.tile_pool(name="ktp", bufs=2, space="PSUM") as ktp_pool,
                tc.tile_pool(name="ptp", bufs=2, space="PSUM") as pt_pool,
                tc.tile_pool(name="opp", bufs=2, space="PSUM") as op_pool,
            ):
                st_load = {}   # idx -> (k_comb, v_f32)
                st_front = {}  # idx -> (kt_sb, v_view)
                st_mid = {}    # idx -> et_sb
                st_out = {}    # idx -> (out_sb, b, w0)

                def emit_loads(idx):
                    b = idx // (w // G)
                    w0 = (idx % (w // G)) * G
                    k_comb = kc_pool.tile([128, G * 128], F32)
                    nc.sync.dma_start(
                        out=k_comb[:].rearrange("p (g j d) -> p g j d", g=G, j=2),
                        in_=k_t[b, w0 : w0 + G].rearrange(
                            "g (p j) d -> p g j d", j=2
                        ),
                    )
                    v_f32 = vf_pool.tile([128, G * 128], F32)
                    nc.sync.dma_start(
                        out=v_f32[:].rearrange("p (g j d) -> p g j d", g=G, j=2),
                        in_=v_t[b, w0 : w0 + G].rearrange(
                            "g (p j) d -> p g j d", j=2
                        ),
                    )
                    st_load[idx] = (k_comb, v_f32)

                def emit_front(idx):
                    # PE transposes; kt PSUM -> SBUF fp16 (DVE + ACT);
                    # V cast (GpSimd pairs 01 + DVE pairs 23)
                    k_comb, v_f32 = st_load.pop(idx)
                    v_ext = vx_pool.tile([128, G * 2 * 65], BF16)
                    v_view = v_ext[:].rearrange(
                        "p (g j c) -> p g j c", g=G, j=2
                    )
                    vf_view = v_f32[:].rearrange(
                        "p (g j d) -> p g j d", g=G, j=2
                    )
                    nc.gpsimd.tensor_copy(
                        v_view[:, 0:2, :, 0:64], vf_view[:, 0:2]
                    )
                    nc.vector.tensor_copy(
                        v_view[:, 2:4, :, 0:64], vf_view[:, 2:4]
                    )
                    nc.gpsimd.memset(v_view[:, :, :, 64:65], 1.0)

                    kt_ps = ktp_pool.tile([128, G * 128], F32)
                    for g in range(G):
                        nc.tensor.matmul(
                            kt_ps[:, g * 128 : (g + 1) * 128],
                            k_comb[:, g * 128 : (g + 1) * 128],
                            ident[:],
                            is_transpose=True,
                            start=(g == 0),
                            stop=(g == G - 1),
                        )
                    kt_sb = kt_pool.tile([128, G * 128], FP16)
                    nc.vector.tensor_copy(kt_sb[:, 0:256], kt_ps[:, 0:256])
                    nc.scalar.copy(kt_sb[:, 256:512], kt_ps[:, 256:512])
                    st_front[idx] = (kt_sb, v_view)

                def emit_mid(idx):
                    # pT fp16 matmuls + one contiguous exp -> bf16
                    kt_sb, _ = st_front[idx]
                    et_sb = et_pool.tile([128, G * 256], BF16)
                    pt_ps = pt_pool.tile([128, G * 256], F32)
                    # bank-alternating order (g0,g2 then g1,g3) so
                    # consecutive matmuls target different PSUM banks
                    for g in (0, 2, 1, 3):
                        nc.tensor.matmul(
                            pt_ps[:, g * 256 : (g + 1) * 256],
                            kt_sb[:, g * 128 : (g + 1) * 128],
                            qz_cat[:],
                            start=(g % 2 == 0),
                            stop=(g % 2 == 1),
                        )
                    nc.scalar.activation(et_sb[:], pt_ps[:], EXP)
                    st_mid[idx] = et_sb

                def emit_out(idx):
                    # out[s, v|den] += E_j.T @ [V_j | 1]; j-major order so
                    # the accumulate RAW chains interleave
                    _, v_view = st_front.pop(idx)
                    et_sb = st_mid.pop(idx)
                    b = idx // (w // G)
                    w0 = (idx % (w // G)) * G
                    out_ps = op_pool.tile([128, G * 65], F32)
                    for j in range(2):
                        for g in range(G):
                            nc.tensor.matmul(
                                out_ps[:, g * 65 : g * 65 + 65],
                                et_sb[:, (2 * g + j) * 128 : (2 * g + j + 1) * 128],
                                v_view[:, g, j, :],
                                start=(g == 0 and j == 0),
                                stop=(g == G - 1 and j == 1),
                            )

                    # normalize (DVE -- GpSimd cannot read PSUM)
                    recip = rc_pool.tile([128, G], F32)
                    out_view = out_ps[:].rearrange("p (g c) -> p g c", c=65)
                    nc.vector.reciprocal(recip[:], out_view[:, :, 64])
                    out_sb = os_pool.tile([128, G * 64], F32)
                    nc.vector.tensor_mul(
                        out_sb[:].rearrange("p (g v) -> p g v", g=G),
                        out_view[:, :, 0:64],
                        recip[:].rearrange("p (g o) -> p g o", o=1).broadcast_to(
                            [128, G, 64]
                        ),
                    )
                    st_out[idx] = (out_sb, b, w0)

                def emit_store(idx):
                    out_sb, b, w0 = st_out.pop(idx)
                    nc.sync.dma_start(
                        out=o_t[b, w0 : w0 + G].rearrange("g s v -> s g v"),
                        in_=out_sb[:].rearrange("p (g v) -> p g v", g=G),
                    )

                for i in range(n_groups + 5):
                    if i < n_groups:
                        emit_loads(i)
                    if 2 <= i < n_groups + 2:
                        emit_mid(i - 2)
                    if 1 <= i < n_groups + 1:
                        emit_front(i - 1)
                    if 3 <= i < n_groups + 3:
                        emit_out(i - 3)
                    if i >= 5:
                        emit_store(i - 5)
    nc.finalize()
    return nc


_NC_CACHE = {}


def _get_nc(b_per=B_PER, w=W):
    key = (b_per, w)
    if key not in _NC_CACHE:
        _NC_CACHE[key] = build_bass(b_per, w)
    return _NC_CACHE[key]


def run(query, keys, values, trace=False):
    query = np.ascontiguousarray(np.asarray(query), dtype=np.float32)
    keys = np.ascontiguousarray(np.asarray(keys), dtype=np.float32)
    values = np.ascontiguousarray(np.asarray(values), dtype=np.float32)
    nc = _get_nc()
    in_maps = [
        {
            "query": query,
            "keys": keys[c * B_PER : (c + 1) * B_PER],
            "values": values[c * B_PER : (c + 1) * B_PER],
        }
        for c in range(N_CORES)
    ]
    res = run_bass_kernel_spmd(nc, in_maps, list(range(N_CORES)), trace=trace)
    out = np.concatenate([res.results[c]["out"] for c in range(N_CORES)], axis=0)
    return out, res


def kernel(query, keys, values):
    out, _ = run(query, keys, values)
    return out


# revision 17
# speedup vs baseline: 1.0126x; 1.0126x over previous
"""Trainium2 Bass kernel for batched shared-query attention.

Problem:
  query [S=128, D=64] shared across all (b, w);
  keys/values [B=64, W=32, T=256, D=64];
  out[b, w] = softmax(query @ keys[b, w].T, axis=-1) @ values[b, w].

Strategy (8 NeuronCores, data-parallel over B), v5 = fp16 score matmuls,
bf16 output matmuls, 512B-chunk loads, engine-balanced elementwise work,
software-pipelined emission:
  Each core gets B_PER=8 batches (256 (b, w) pairs), G=4 pairs per group.
    1. K and V both loaded t-pair-interleaved: sbuf [128, G*128] fp32,
       partition p holds rows t=2p, 2p+1 (512B contiguous DMA chunks --
       512B packets run ~18 B/ns/engine vs ~15 for 256B).
    2. V fp32 -> bf16 cast into [128, G*2*65] with a ones column per
       (pair, t-parity) block (the 65th rhs column makes the output
       matmul emit the softmax denominator for free).  Cast is split:
       pairs 01 on GpSimd (its only possible job -- it cannot touch
       PSUM), pairs 23 on DVE.
    3. PE transpose of K (fp32) -> kt PSUM; PSUM -> SBUF fp16 copy split
       half on DVE / half on ACT (stacked Kt: partitions 0:64 = K^T of
       even t, 64:128 = odd t).  fp16 (not bf16): the 10-bit mantissa
       keeps softmax weight error ~0.4% (bf16 scores fail the 2e-2 gate).
    4. pT = stacked-Kt.T @ qz_cat, ONE fp16 matmul per pair (N=256,
       1 cyc/row): qz_cat [128, 256] fp16 holds Qt twice, zero-padded
       parity-blocked, so both t-parities come out side by side.
    5. ACT exp: ONE plain [128, 1024] PSUM -> SBUF bf16 instr per group
       (bf16 output: exp range exceeds fp16 max).
    6. Output matmuls, m=128: per (pair, t-parity j):
       out[s, v|den] += E_j.T @ [V_j | 1] (N=65, bf16), 8 per group into
       one PSUM tile [128, 260] with column-disjoint [128, 65] regions
       (single 2KB bank; first/last matmul carries start/stop).
    7. DVE reciprocal of the 4 denominators + one broadcast multiply;
       store DMA (256B chunks -- the s-major layout costs ~3% DMA time,
       much less than the extra matmuls/normalize of a 512B-store layout).
  Emission is software-pipelined across groups (loads at i, stages at
  i-1/i-2/i-3, store at i-5) so every engine's queue head is always
  ready: the PE never stalls (keeps its p-state ramped) and the store's
  semaphore wait cannot delay the next loads on the SP queue.
  All matmuls keep full 128-row contractions at tile_position (0,0) --
  alternating row-group (K=64) matmuls fault on HW.
"""

import sys

sys.path.insert(0, "/opt/trn_rl_repo")

import numpy as np

import concourse.bass as bass
from concourse import bacc
import concourse.mybir as mybir
import concourse.tile as tile
from concourse.bass_utils import run_bass_kernel_spmd
from concourse.masks import make_identity

F32 = mybir.dt.float32
BF16 = mybir.dt.bfloat16
FP16 = mybir.dt.float16
N_CORES = 8
B, W, T, S, D = 64, 32, 256, 128, 64
B_PER = B // N_CORES
G = 4  # (b, w) pairs per group


def build_bass(b_per=B_PER, w=W):
    nc = bacc.Bacc()
    q_t = nc.declare_dram_parameter("query", [S, D], F32, isOutput=False)
    k_t = nc.declare_dram_parameter("keys", [b_per, w, T, D], F32, isOutput=False)
    v_t = nc.declare_dram_parameter("values", [b_per, w, T, D], F32, isOutput=False)
    o_t = nc.declare_dram_parameter("out", [b_per, w, S, D], F32, isOutput=True)

    EXP = mybir.ActivationFunctionType.Exp

    with tile.TileContext(nc) as tc:
        with tc.tile_pool(name="const", bufs=1) as const:
            ident = const.tile([128, 128], F32)
            make_identity(nc, ident[:])
            q_sb = const.tile([S, D], F32)
            nc.sync.dma_start(out=q_sb[:], in_=q_t[:, :])
            # qz_cat [128, 256] fp16:
            #   rows 0:64,   cols   0:128 = Qt  (contracts Kt_even rows)
            #   rows 64:128, cols 128:256 = Qt  (contracts Kt_odd rows)
            #   everything else 0.
            qz_cat = const.tile([128, 2 * S], FP16)
            nc.vector.memset(qz_cat[:], 0.0)
            with tc.tile_pool(name="psetup", bufs=1, space="PSUM") as psetup:
                qt_ps = psetup.tile([64, S], F32)
                nc.tensor.matmul(
                    qt_ps[:, :], q_sb[:], ident[:],
                    is_transpose=True, start=True, stop=True,
                )
                nc.scalar.copy(qz_cat[0:64, 0:S], qt_ps[:])
            # place Qt on partitions 64:128 via a DRAM roundtrip
            # (cross-partition engine copies are not available)
            qt_scratch = nc.dram_tensor("qt_scratch", [64, S], FP16)
            nc.sync.dma_start(out=qt_scratch[:, :], in_=qz_cat[0:64, 0:S])
            nc.sync.dma_start(out=qz_cat[64:128, S : 2 * S], in_=qt_scratch[:, :])

            n_groups = b_per * (w // G)

            with (
                tc.tile_pool(name="kc", bufs=3) as kc_pool,
                tc.tile_pool(name="vf", bufs=3) as vf_pool,
                tc.tile_pool(name="vx", bufs=4) as vx_pool,
                tc.tile_pool(name="kts", bufs=3) as kt_pool,
                tc.tile_pool(name="et", bufs=3) as et_pool,
                tc.tile_pool(name="osb", bufs=4) as os_pool,
                tc.tile_pool(name="rc", bufs=4) as rc_pool,
                tc.tile_pool(name="ktp", bufs=2, space="PSUM") as ktp_pool,
                tc.tile_pool(name="ptp", bufs=2, space="PSUM") as pt_pool,
                tc.tile_pool(name="opp", bufs=2, space="PSUM") as op_pool,
            ):
                st_load = {}   # idx -> (k_comb, v_f32)
                st_front = {}  # idx -> (kt_sb, v_view)
                st_mid = {}    # idx -> et_sb
                st_out = {}    # idx -> (out_sb, b, w0)

                def emit_loads(idx):
                    b = idx // (w // G)
                    w0 = (idx % (w // G)) * G
                    k_comb = kc_pool.tile([128, G * 128], F32)
                    nc.sync.dma_start(
                        out=k_comb[:].rearrange("p (g j d) -> p g j d", g=G, j=2),
                        in_=k_t[b, w0 : w0 + G].rearrange(
                            "g (p j) d -> p g j d", j=2
                        ),
                    )
                    v_f32 = vf_pool.tile([128, G * 128], F32)
                    nc.sync.dma_start(
                        out=v_f32[:].rearrange("p (g j d) -> p g j d", g=G, j=2),
                        in_=v_t[b, w0 : w0 + G].rearrange(
                            "g (p j) d -> p g j d", j=2
                        ),
                    )
                    st_load[idx] = (k_comb, v_f32)

                def emit_front(idx):
                    # PE transposes; kt PSUM -> SBUF fp16 (DVE + ACT);
                    # V cast (GpSimd pairs 01 + DVE pairs 23)
                    k_comb, v_f32 = st_load.pop(idx)
                    v_ext = vx_pool.tile([128, G * 2 * 65], BF16)
                    v_view = v_ext[:].rearrange(
                        "p (g j c) -> p g j c", g=G, j=2
                    )
                    vf_view = v_f32[:].rearrange(
                        "p (g j d) -> p g j d", g=G, j=2
                    )
                    nc.gpsimd.tensor_copy(
                        v_view[:, 0:2, :, 0:64], vf_view[:, 0:2]
                    )
                    nc.vector.tensor_copy(
                        v_view[:, 2:4, :, 0:64], vf_view[:, 2:4]
                    )
                    nc.gpsimd.memset(v_view[:, :, :, 64:65], 1.0)

                    kt_ps = ktp_pool.tile([128, G * 128], F32)
                    for g in range(G):
                        nc.tensor.matmul(
                            kt_ps[:, g * 128 : (g + 1) * 128],
                            k_comb[:, g * 128 : (g + 1) * 128],
                            ident[:],
                            is_transpose=True,
                            start=(g == 0),
                            stop=(g == G - 1),
                        )
                    kt_sb = kt_pool.tile([128, G * 128], FP16)
                    nc.vector.tensor_copy(kt_sb[:, 0:256], kt_ps[:, 0:256])
                    nc.scalar.copy(kt_sb[:, 256:512], kt_ps[:, 256:512])
                    st_front[idx] = (kt_sb, v_view)

                def emit_mid(idx):
                    # pT fp16 matmuls + one contiguous exp -> bf16
                    kt_sb, _ = st_front[idx]
                    et_sb = et_pool.tile([128, G * 256], BF16)
                    pt_ps = pt_pool.tile([128, G * 256], F32)
                    # bank-alternating order (g0,g2 then g1,g3) so
                    # consecutive matmuls target different PSUM banks
                    for g in (0, 2, 1, 3):
                        nc.tensor.matmul(
                            pt_ps[:, g * 256 : (g + 1) * 256],
                            kt_sb[:, g * 128 : (g + 1) * 128],
                            qz_cat[:],
                            start=(g % 2 == 0),
                            stop=(g % 2 == 1),
                        )
                    nc.scalar.activation(et_sb[:], pt_ps[:], EXP)
                    st_mid[idx] = et_sb

                def emit_out(idx):
                    # out[s, v|den] += E_j.T @ [V_j | 1]; j-major order so
                    # the accumulate RAW chains interleave
                    _, v_view = st_front.pop(idx)
                    et_sb = st_mid.pop(idx)
                    b = idx // (w // G)
                    w0 = (idx % (w // G)) * G
                    out_ps = op_pool.tile([128, G * 65], F32)
                    for j in range(2):
                        for g in range(G):
                            nc.tensor.matmul(
                                out_ps[:, g * 65 : g * 65 + 65],
                                et_sb[:, (2 * g + j) * 128 : (2 * g + j + 1) * 128],
                                v_view[:, g, j, :],
                                start=(g == 0 and j == 0),
                                stop=(g == G - 1 and j == 1),
                            )

                    # normalize (DVE -- GpSimd cannot read PSUM)
                    recip = rc_pool.tile([128, G], F32)
                    out_view = out_ps[:].rearrange("p (g c) -> p g c", c=65)
                    nc.vector.reciprocal(recip[:], out_view[:, :, 64])
                    out_sb = os_pool.tile([128, G * 64], F32)
                    nc.vector.tensor_mul(
                        out_sb[:].rearrange("p (g v) -> p g v", g=G),
                        out_view[:, :, 0:64],
                        recip[:].rearrange("p (g o) -> p g o", o=1).broadcast_to(
                            [128, G, 64]
                        ),
                    )
                    st_out[idx] = (out_sb, b, w0)

                def emit_store(idx):
                    out_sb, b, w0 = st_out.pop(idx)
                    nc.sync.dma_start(
                        out=o_t[b, w0 : w0 + G].rearrange("g s v -> s g v"),
                        in_=out_sb[:].rearrange("p (g v) -> p g v", g=G),
                    )

                for i in range(n_groups + 4):
                    if i < n_groups:
                        emit_loads(i)
                    if 2 <= i < n_groups + 2:
                        emit_mid(i - 2)
                    if 1 <= i < n_groups + 1:
                        emit_front(i - 1)
                    if 3 <= i < n_groups + 3:
                        emit_out(i - 3)
                    if i >= 4:
                        emit_store(i - 4)
    nc.finalize()
    return nc


_NC_CACHE = {}


def _get_nc(b_per=B_PER, w=W):
    key = (b_per, w)
    if key not in _NC_CACHE:
        _NC_CACHE[key] = build_bass(b_per, w)
    return _NC_CACHE[key]


def run(query, keys, values, trace=False):
    query = np.ascontiguousarray(np.asarray(query), dtype=np.float32)
    keys = np.ascontiguousarray(np.asarray(keys), dtype=np.float32)
    values = np.ascontiguousarray(np.asarray(values), dtype=np.float32)
    nc = _get_nc()
    in_maps = [
        {
            "query": query,
            "keys": keys[c * B_PER : (c + 1) * B_PER],
            "values": values[c * B_PER : (c + 1) * B_PER],
        }
        for c in range(N_CORES)
    ]
    res = run_bass_kernel_spmd(nc, in_maps, list(range(N_CORES)), trace=trace)
    out = np.concatenate([res.results[c]["out"] for c in range(N_CORES)], axis=0)
    return out, res


def kernel(query, keys, values):
    out, _ = run(query, keys, values)
    return out


# revision 18
# speedup vs baseline: 1.0483x; 1.0353x over previous
"""Trainium2 Bass kernel for batched shared-query attention.

Problem:
  query [S=128, D=64] shared across all (b, w);
  keys/values [B=64, W=32, T=256, D=64];
  out[b, w] = softmax(query @ keys[b, w].T, axis=-1) @ values[b, w].

Strategy (8 NeuronCores, data-parallel over B), v5 = fp16 score matmuls,
bf16 output matmuls, 512B-chunk loads, engine-balanced elementwise work,
software-pipelined emission:
  Each core gets B_PER=8 batches (256 (b, w) pairs), G=4 pairs per group.
    1. K and V both loaded t-pair-interleaved: sbuf [128, G*128] fp32,
       partition p holds rows t=2p, 2p+1 (512B contiguous DMA chunks --
       512B packets run ~18 B/ns/engine vs ~15 for 256B).
    2. V fp32 -> bf16 cast into [128, G*2*65] with a ones column per
       (pair, t-parity) block (the 65th rhs column makes the output
       matmul emit the softmax denominator for free).  Cast is split:
       pairs 01 on GpSimd (its only possible job -- it cannot touch
       PSUM), pairs 23 on DVE.
    3. PE transpose of K (fp32) -> kt PSUM; PSUM -> SBUF fp16 copy split
       half on DVE / half on ACT (stacked Kt: partitions 0:64 = K^T of
       even t, 64:128 = odd t).  fp16 (not bf16): the 10-bit mantissa
       keeps softmax weight error ~0.4% (bf16 scores fail the 2e-2 gate).
    4. pT = stacked-Kt.T @ qz_cat, ONE fp16 matmul per pair (N=256,
       1 cyc/row): qz_cat [128, 256] fp16 holds Qt twice, zero-padded
       parity-blocked, so both t-parities come out side by side.
    5. ACT exp: ONE plain [128, 1024] PSUM -> SBUF bf16 instr per group
       (bf16 output: exp range exceeds fp16 max).
    6. Output matmuls, m=128: per (pair, t-parity j):
       out[s, v|den] += E_j.T @ [V_j | 1] (N=65, bf16), 8 per group into
       one PSUM tile [128, 260] with column-disjoint [128, 65] regions
       (single 2KB bank; first/last matmul carries start/stop).
    7. DVE reciprocal of the 4 denominators + one broadcast multiply;
       store DMA (256B chunks -- the s-major layout costs ~3% DMA time,
       much less than the extra matmuls/normalize of a 512B-store layout).
  Emission is software-pipelined across groups (loads at i, stages at
  i-1/i-2/i-3, store at i-5) so every engine's queue head is always
  ready: the PE never stalls (keeps its p-state ramped) and the store's
  semaphore wait cannot delay the next loads on the SP queue.
  All matmuls keep full 128-row contractions at tile_position (0,0) --
  alternating row-group (K=64) matmuls fault on HW.
"""

import sys

sys.path.insert(0, "/opt/trn_rl_repo")

import numpy as np

import concourse.bass as bass
from concourse import bacc
import concourse.mybir as mybir
import concourse.tile as tile
from concourse.bass_utils import run_bass_kernel_spmd
from concourse.masks import make_identity

F32 = mybir.dt.float32
BF16 = mybir.dt.bfloat16
FP16 = mybir.dt.float16
N_CORES = 8
B, W, T, S, D = 64, 32, 256, 128, 64
B_PER = B // N_CORES
G = 4  # (b, w) pairs per group


def build_bass(b_per=B_PER, w=W):
    nc = bacc.Bacc()
    q_t = nc.declare_dram_parameter("query", [S, D], F32, isOutput=False)
    k_t = nc.declare_dram_parameter("keys", [b_per, w, T, D], F32, isOutput=False)
    v_t = nc.declare_dram_parameter("values", [b_per, w, T, D], F32, isOutput=False)
    o_t = nc.declare_dram_parameter("out", [b_per, w, S, D], F32, isOutput=True)

    EXP = mybir.ActivationFunctionType.Exp

    with tile.TileContext(nc) as tc:
        with tc.tile_pool(name="const", bufs=1) as const:
            ident = const.tile([128, 128], F32)
            make_identity(nc, ident[:])
            q_sb = const.tile([S, D], F32)
            nc.sync.dma_start(out=q_sb[:], in_=q_t[:, :])
            # qz_cat [128, 256] fp16:
            #   rows 0:64,   cols   0:128 = Qt  (contracts Kt_even rows)
            #   rows 64:128, cols 128:256 = Qt  (contracts Kt_odd rows)
            #   everything else 0.
            qz_cat = const.tile([128, 2 * S], FP16)
            nc.vector.memset(qz_cat[:], 0.0)
            with tc.tile_pool(name="psetup", bufs=1, space="PSUM") as psetup:
                qt_ps = psetup.tile([64, S], F32)
                nc.tensor.matmul(
                    qt_ps[:, :], q_sb[:], ident[:],
                    is_transpose=True, start=True, stop=True,
                )
                nc.scalar.copy(qz_cat[0:64, 0:S], qt_ps[:])
            # place Qt on partitions 64:128 via a DRAM roundtrip
            # (cross-partition engine copies are not available)
            qt_scratch = nc.dram_tensor("qt_scratch", [64, S], FP16)
            nc.sync.dma_start(out=qt_scratch[:, :], in_=qz_cat[0:64, 0:S])
            nc.sync.dma_start(out=qz_cat[64:128, S : 2 * S], in_=qt_scratch[:, :])

            n_groups = b_per * (w // G)

            with (
                tc.tile_pool(name="kc", bufs=3) as kc_pool,
                tc.tile_pool(name="vf", bufs=3) as vf_pool,
                tc.tile_pool(name="vx", bufs=4) as vx_pool,
                tc.tile_pool(name="kts", bufs=3) as kt_pool,
                tc.tile_pool(name="et", bufs=3) as et_pool,
                tc.tile_pool(name="osb", bufs=4) as os_pool,
                tc.tile_pool(name="rc", bufs=4) as rc_pool,
                tc.tile_pool(name="ktp", bufs=2, space="PSUM") as ktp_pool,
                tc.tile_pool(name="ptp", bufs=2, space="PSUM") as pt_pool,
                tc.tile_pool(name="opp", bufs=2, space="PSUM") as op_pool,
            ):
                st_load = {}   # idx -> (k_comb, v_f32)
                st_front = {}  # idx -> (kt_sb, v_view)
                st_mid = {}    # idx -> et_sb
                st_out = {}    # idx -> (out_sb, b, w0)

                def emit_loads(idx):
                    b = idx // (w // G)
                    w0 = (idx % (w // G)) * G
                    k_comb = kc_pool.tile([128, G * 128], F32)
                    nc.sync.dma_start(
                        out=k_comb[:].rearrange("p (g j d) -> p g j d", g=G, j=2),
                        in_=k_t[b, w0 : w0 + G].rearrange(
                            "g (p j) d -> p g j d", j=2
                        ),
                    )
                    v_f32 = vf_pool.tile([128, G * 128], F32)
                    nc.sync.dma_start(
                        out=v_f32[:].rearrange("p (g j d) -> p g j d", g=G, j=2),
                        in_=v_t[b, w0 : w0 + G].rearrange(
                            "g (p j) d -> p g j d", j=2
                        ),
                    )
                    st_load[idx] = (k_comb, v_f32)

                def emit_front(idx):
                    # PE transposes; kt PSUM -> SBUF fp16 (DVE + ACT);
                    # V cast (GpSimd pairs 01 + DVE pairs 23)
                    k_comb, v_f32 = st_load.pop(idx)
                    v_ext = vx_pool.tile([128, G * 2 * 65], BF16)
                    v_view = v_ext[:].rearrange(
                        "p (g j c) -> p g j c", g=G, j=2
                    )
                    vf_view = v_f32[:].rearrange(
                        "p (g j d) -> p g j d", g=G, j=2
                    )
                    nc.gpsimd.tensor_copy(
                        v_view[:, 0:2, :, 0:64], vf_view[:, 0:2]
                    )
                    nc.vector.tensor_copy(
                        v_view[:, 2:4, :, 0:64], vf_view[:, 2:4]
                    )
                    nc.gpsimd.memset(v_view[:, :, :, 64:65], 1.0)

                    kt_ps = ktp_pool.tile([128, G * 128], F32)
                    for g in range(G):
                        nc.tensor.matmul(
                            kt_ps[:, g * 128 : (g + 1) * 128],
                            k_comb[:, g * 128 : (g + 1) * 128],
                            ident[:],
                            is_transpose=True,
                            start=(g == 0),
                            stop=(g == G - 1),
                        )
                    kt_sb = kt_pool.tile([128, G * 128], FP16)
                    nc.vector.tensor_copy(kt_sb[:, 0:256], kt_ps[:, 0:256])
                    nc.scalar.copy(kt_sb[:, 256:512], kt_ps[:, 256:512])
                    st_front[idx] = (kt_sb, v_view)

                def emit_mid(idx):
                    # pT fp16 matmuls + one contiguous exp -> bf16
                    kt_sb, _ = st_front[idx]
                    et_sb = et_pool.tile([128, G * 256], BF16)
                    pt_ps = pt_pool.tile([128, G * 256], F32)
                    # bank-alternating order (g0,g2 then g1,g3) so
                    # consecutive matmuls target different PSUM banks
                    for g in (0, 2, 1, 3):
                        nc.tensor.matmul(
                            pt_ps[:, g * 256 : (g + 1) * 256],
                            kt_sb[:, g * 128 : (g + 1) * 128],
                            qz_cat[:],
                            start=(g % 2 == 0),
                            stop=(g % 2 == 1),
                        )
                    nc.scalar.activation(et_sb[:], pt_ps[:], EXP)
                    st_mid[idx] = et_sb

                def emit_out(idx):
                    # out[s, v|den] += E_j.T @ [V_j | 1]; j-major order so
                    # the accumulate RAW chains interleave
                    _, v_view = st_front.pop(idx)
                    et_sb = st_mid.pop(idx)
                    b = idx // (w // G)
                    w0 = (idx % (w // G)) * G
                    out_ps = op_pool.tile([128, G * 65], F32)
                    for j in range(2):
                        for g in range(G):
                            nc.tensor.matmul(
                                out_ps[:, g * 65 : g * 65 + 65],
                                et_sb[:, (2 * g + j) * 128 : (2 * g + j + 1) * 128],
                                v_view[:, g, j, :],
                                start=(g == 0 and j == 0),
                                stop=(g == G - 1 and j == 1),
                            )

                    # normalize (DVE -- GpSimd cannot read PSUM)
                    recip = rc_pool.tile([128, G], F32)
                    out_view = out_ps[:].rearrange("p (g c) -> p g c", c=65)
                    nc.vector.reciprocal(recip[:], out_view[:, :, 64])
                    out_sb = os_pool.tile([128, G * 64], F32)
                    nc.vector.tensor_mul(
                        out_sb[:].rearrange("p (g v) -> p g v", g=G),
                        out_view[:, :, 0:64],
                        recip[:].rearrange("p (g o) -> p g o", o=1).broadcast_to(
                            [128, G, 64]
                        ),
                    )
                    st_out[idx] = (out_sb, b, w0)

                def emit_store(idx):
                    out_sb, b, w0 = st_out.pop(idx)
                    nc.sync.dma_start(
                        out=o_t[b, w0 : w0 + G].rearrange("g s v -> s g v"),
                        in_=out_sb[:].rearrange("p (g v) -> p g v", g=G),
                    )

                for i in range(n_groups + 5):
                    if i < n_groups:
                        emit_loads(i)
                    if 2 <= i < n_groups + 2:
                        emit_mid(i - 2)
                    if 1 <= i < n_groups + 1:
                        emit_front(i - 1)
                    if 3 <= i < n_groups + 3:
                        emit_out(i - 3)
                    if i >= 5:
                        emit_store(i - 5)
    nc.finalize()
    return nc


_NC_CACHE = {}


def _get_nc(b_per=B_PER, w=W):
    key = (b_per, w)
    if key not in _NC_CACHE:
        _NC_CACHE[key] = build_bass(b_per, w)
    return _NC_CACHE[key]


def run(query, keys, values, trace=False):
    query = np.ascontiguousarray(np.asarray(query), dtype=np.float32)
    keys = np.ascontiguousarray(np.asarray(keys), dtype=np.float32)
    values = np.ascontiguousarray(np.asarray(values), dtype=np.float32)
    nc = _get_nc()
    in_maps = [
        {
            "query": query,
            "keys": keys[c * B_PER : (c + 1) * B_PER],
            "values": values[c * B_PER : (c + 1) * B_PER],
        }
        for c in range(N_CORES)
    ]
    res = run_bass_kernel_spmd(nc, in_maps, list(range(N_CORES)), trace=trace)
    out = np.concatenate([res.results[c]["out"] for c in range(N_CORES)], axis=0)
    return out, res


def kernel(query, keys, values):
    out, _ = run(query, keys, values)
    return out
